# revision 17
# baseline (speedup 1.0000x reference)
"""Distributed multi-head attention kernel for 8 TRN2 NeuronCores.

Sharding: 8-way head parallel (2 heads per core), batches looped on-core.

Design (v3):
- attn@V is V-stationary: out^T[d(+denom), i] = v_aug[j, 65].T @ ptile[j, i]
  accumulated over 16 j-chunks with N=512 streaming (vs 1024 tiny N=65
  matmuls in v1 that were LDWEIGHTS-bound at 240us of PE time).
- QK^T dots are ROW-TILED: qT/kT are packed [h*64+d, i] (head 0 in
  partitions 0-63, head 1 in 64-127) and the two heads' dots run
  CONCURRENTLY on the PE's 64-row sub-arrays (verified ~4ns apart on HW).
- The softmax exp stream on the Scalar engine (128 ACTIVATEs x ~1.35us =
  173us) is the roofline wall; every other engine is scheduled around
  keeping it fed.  Engines are FIFO, so PE work is emitted in fine-grained
  slots: each j-chunk slot emits [ST pair][one pending closure][one filler
  unit].  Pending closures carry attn@V pairs 4 slots behind their exp
  (so they never head-of-line block on ACT) and the normalize+staging of
  the previous block; filler units are 8-matmul chunks of the qkv
  projections for batch 1 and the batch-0 output projection.
- Normalize: psO row 64 holds the denominators (ones-column of v_aug).
  psO is copied to SBUF (unnorm + denom rows) to free the single psO
  buffer quickly, then reciprocal -> rank-1 PE broadcast (ones[1,64].T @
  recip) into a q_psum bank -> one DVE multiply per head (SBUF x PSUM).
- Collectives have a ~28us ncfw latency floor regardless of size, so only
  3 AllToAlls: batch 0 (fires at half-time, hidden), blocks (1,0)+(1,1)
  (hidden under the last blocks), (1,2)+(1,3) in the tail.  The gpsimd
  queue carries ONLY staging DMAs + collective triggers (a collective
  trigger blocks the gpsimd engine until completion); receives ride sync.
- The [d, i] output orientation makes receives pure DMA (no transposes);
  the x prologue streams over 3 DMA queues so the first exp fires ~18us in.

The per-core output is the TRANSPOSED final slice [1024, 8, 64] (cols
keyed by (batch, ib, 64-row chunk)); the host transposes during assembly.
"""
from collections import deque

import numpy as np

import concourse.bass as bass
import concourse.mybir as mybir
from concourse import bacc
import concourse.tile as tile
from concourse.bass_utils import run_bass_kernel_spmd

# problem constants (hardcoded; kernel.py must be self-contained)
B, N, DIM = 2, 2048, 1024
H, DH = 16, 64
INNER = H * DH            # 1024
SCALE = DIM ** -0.5       # 1/32  (module scales by dim**-0.5, not dim_head)
NCORES = 8
HPC = H // NCORES         # 2 heads per core
SH = HPC * DH             # 128 inner cols per core
P = 128
KO = DIM // P             # 8 contraction chunks
JC = N // P               # 16 key chunks
IB = 512                  # query block size
NIB = N // IB             # 4 query blocks per batch
NCOLL = B * NIB           # 8 (batch, query-block) output blocks
RPC = IB // NCORES        # 64 rows per core per block
FP32 = mybir.dt.float32
BF16 = mybir.dt.bfloat16

REPLICA_GROUPS = [[0, 1, 2, 3, 4, 5, 6, 7]]
# collective -> list of (b, ib) blocks it carries
COLLS = [[(0, 0), (0, 1), (0, 2), (0, 3)], [(1, 0), (1, 1)], [(1, 2), (1, 3)]]

_NC_CACHE = {}

# set by the last kernel() call when BASS_KERNEL_TRACE=1 (for test.py)
LAST_RESULTS = None


def _build():
    nc = bacc.Bacc(num_devices=NCORES)

    x_ext = nc.declare_dram_parameter("x", [B * DIM, N], BF16, isOutput=False)
    wq_ext = nc.declare_dram_parameter("wq", [DIM, SH], BF16, isOutput=False)
    wk_ext = nc.declare_dram_parameter("wk", [DIM, SH], BF16, isOutput=False)
    wv_ext = nc.declare_dram_parameter("wv", [DIM, SH], BF16, isOutput=False)
    wo_ext = nc.declare_dram_parameter("wo", [DIM, DIM], BF16, isOutput=False)
    bo_ext = nc.declare_dram_parameter("bo", [DIM], FP32, isOutput=False)
    out_ext = nc.declare_dram_parameter(
        "out", [DIM, NCOLL, RPC], FP32, isOutput=True
    )

    with tile.TileContext(nc) as tc:
        with (
            tc.tile_pool(name="consts", bufs=1) as consts,
            tc.tile_pool(name="stage", bufs=2) as stage,
            tc.tile_pool(name="xt_pool", bufs=2) as xt_pool,
            tc.tile_pool(name="pt_pool", bufs=2) as pt_pool,
            tc.tile_pool(name="nrm", bufs=1) as nrm,
            tc.tile_pool(name="unm", bufs=1) as unm,
            tc.tile_pool(name="dram", bufs=1, space="DRAM") as dram,
            tc.tile_pool(name="st_psum", bufs=2, space="PSUM") as st_psum,
            tc.tile_pool(name="o_psum", bufs=1, space="PSUM") as o_psum,
            tc.tile_pool(name="q_psum", bufs=2, space="PSUM") as q_psum,
        ):
            # persistent tensors
            wq_sb = consts.tile([P, KO, SH], BF16)
            wk_sb = consts.tile([P, KO, SH], BF16)
            wv_sb = consts.tile([P, KO, SH], BF16)
            wo_sb = consts.tile([P, KO, DIM], BF16)
            bias_sb = consts.tile([P, KO], FP32)
            ones64 = consts.tile([1, DH], BF16)
            # packed [h*64+d, i]: head h of this core in partitions h*64..
            qT = consts.tile([P, B, N], BF16)
            kT = consts.tile([P, B, N], BF16)
            v_aug = consts.tile([P, B, JC, HPC, DH + 1], BF16)
            # normalized transposed attention output [h*64+d, b, ib, c, i]
            outT = consts.tile([P, B, NIB, NCORES, RPC], BF16)
            # received: [inner mod 128, src core(=inner/128), block, row]
            attnT = consts.tile([P, KO, NCOLL, RPC], BF16)

            a2a_ins = [
                dram.tile([NCORES, P, len(blks), RPC], BF16, name=f"a2a_in{k}")
                for k, blks in enumerate(COLLS)
            ]
            a2a_outs = [
                dram.tile([NCORES, P, len(blks), RPC], BF16, name=f"a2a_out{k}")
                for k, blks in enumerate(COLLS)
            ]

            # preload the exp table right away (dummy activation on ones64)
            nc.vector.memset(ones64, 1.0)
            warm = consts.tile([1, 2], FP32)
            nc.scalar.activation(
                warm, ones64[:, 0:2], mybir.ActivationFunctionType.Exp
            )
            nc.vector.memset(v_aug[:, :, :, :, DH : DH + 1], 1.0)

            # ---- input DMAs spread over 3 queues (sync/scalar/gpsimd) ----
            def load_x_nb(b, nb, xT, eng):
                for ko in range(KO):
                    eng.dma_start(
                        xT[:, ko, nb * IB : (nb + 1) * IB],
                        x_ext[
                            b * DIM + ko * P : b * DIM + (ko + 1) * P,
                            nb * IB : (nb + 1) * IB,
                        ],
                    )

            def load_w(w_ext, w_sb):
                nc.scalar.dma_start(
                    w_sb, w_ext.rearrange("(ko kp) c -> kp ko c", kp=P)
                )

            xT0 = xt_pool.tile([P, KO, N], BF16, tag="xT", name="xT0")
            xT1 = xt_pool.tile([P, KO, N], BF16, tag="xT", name="xT1")
            load_w(wk_ext, wk_sb)
            load_w(wq_ext, wq_sb)
            load_x_nb(0, 0, xT0, nc.sync)
            load_x_nb(0, 1, xT0, nc.scalar)
            load_x_nb(0, 2, xT0, nc.gpsimd)
            load_x_nb(0, 3, xT0, nc.sync)
            load_w(wv_ext, wv_sb)
            load_x_nb(1, 0, xT1, nc.sync)
            load_x_nb(1, 1, xT1, nc.scalar)
            load_x_nb(1, 2, xT1, nc.gpsimd)
            load_x_nb(1, 3, xT1, nc.sync)
            load_w(wo_ext, wo_sb)
            nc.scalar.dma_start(
                bias_sb, bo_ext.rearrange("(co cp) -> cp co", cp=P)
            )

            # warm the PE (HAM un-throttle) while x streams in: ~14 junk
            # matmuls with no input dependencies
            junk = consts.tile([P, IB], BF16)
            nc.vector.memset(junk[:, 0:1], 0.0)
            nc.vector.memset(junk, 0.125)
            for _ in range(14):
                ps_w = q_psum.tile([P, IB], FP32, tag="qk", name="warm_ps")
                nc.tensor.matmul(
                    ps_w, junk[:, 0:P], junk, start=True, stop=True
                )

            # ---- filler units (each ~8 matmuls + a DVE drain) ----
            def qk_unit(b, xT, w_sb, dstT, nb):
                ps = q_psum.tile([P, IB], FP32, tag="qk", name="qk_ps")
                for ko in range(KO):
                    nc.tensor.matmul(
                        ps,
                        w_sb[:, ko, :],
                        xT[:, ko, nb * IB : (nb + 1) * IB],
                        start=(ko == 0),
                        stop=(ko == KO - 1),
                    )
                nc.vector.tensor_copy(dstT[:, b, nb * IB : (nb + 1) * IB], ps)

            def v_unit(b, xT, mt):
                ps_v = q_psum.tile([P, SH], FP32, tag="qk", name="v_ps")
                for ko in range(KO):
                    nc.tensor.matmul(
                        ps_v,
                        xT[:, ko, mt * P : (mt + 1) * P],
                        wv_sb[:, ko, :],
                        start=(ko == 0),
                        stop=(ko == KO - 1),
                    )
                nc.vector.tensor_copy(
                    v_aug[:, b, mt, :, 0:DH],
                    ps_v.rearrange("p (h d) -> p h d", d=DH),
                )

            def final_unit(k, cc):
                """Output projection for collective k's blocks, one 128-col
                chunk of DIM; bias add + result DMA (sync queue)."""
                b0 = 4 * k if k < 2 else 6
                nb = len(COLLS[k])
                csl = slice(b0, b0 + nb)
                ps_f4 = q_psum.tile([P, NIB, RPC], FP32, tag="qk", name="f_ps")
                ps_f = ps_f4[:, 0:nb, :]
                for ko in range(KO):
                    nc.tensor.matmul(
                        ps_f,
                        wo_sb[:, ko, cc * P : (cc + 1) * P],
                        attnT[:, ko, csl, :],
                        start=(ko == 0),
                        stop=(ko == KO - 1),
                    )
                of4 = stage.tile([P, NIB, RPC], FP32, tag="of", name="of")
                of = of4[:, 0:nb, :]
                nc.vector.tensor_scalar_add(of, ps_f, bias_sb[:, cc : cc + 1])
                nc.sync.dma_start(out_ext[cc * P : (cc + 1) * P, csl, :], of)

            # ---- attention blocks with slot-based emission ----
            pending = deque()  # closures: attn@V pairs (trailing) + normalize

            def attnv_closure(b, ib, jc, ptile, psO):
                def emit():
                    for h in range(HPC):
                        nc.tensor.matmul(
                            psO[0 : DH + 1, h, :],
                            v_aug[:, b, jc, h, :],
                            ptile[:, jc, h, :],
                            start=(jc == 0),
                            stop=(jc == JC - 1),
                        )
                return emit

            def norm_a_closure(psO, box):
                def emit():
                    # free psO fast: copy unnormalized rows + denominators out
                    unnorm = unm.tile([DH, HPC, IB], BF16, tag="un", name="un")
                    dnm = nrm.tile([1, HPC, IB], FP32, tag="dn", name="dn")
                    nc.vector.tensor_copy(unnorm, psO[0:DH, :, :])
                    nc.vector.tensor_copy(dnm, psO[DH : DH + 1, :, :])
                    recip = nrm.tile([1, HPC, IB], FP32, tag="rc", name="rc")
                    nc.vector.reciprocal(recip, dnm)
                    rc_bf = nrm.tile([1, HPC, IB], BF16, tag="rcb", name="rcb")
                    nc.vector.tensor_copy(rc_bf, recip)
                    box.append((unnorm, rc_bf))
                return emit

            def norm_b_closure(b, ib, box, coll=None):
                def emit():
                    unnorm, rc_bf = box.pop()
                    for h in range(HPC):
                        psB = q_psum.tile([P, IB], FP32, tag="qk", name="b_ps")
                        nc.tensor.matmul(
                            psB[0:DH, :],
                            ones64,
                            rc_bf[:, h, :],
                            start=True,
                            stop=True,
                        )
                        nc.vector.tensor_tensor(
                            outT[h * DH : (h + 1) * DH, b, ib, :, :],
                            unnorm[:, h, :].rearrange(
                                "d (c i) -> d c i", c=NCORES
                            ),
                            psB[0:DH, :].rearrange("d (c i) -> d c i", c=NCORES),
                            mybir.AluOpType.mult,
                        )
                    # stage this block into its collective's buffer
                    k, slot = coll_of[(b, ib)]
                    nc.gpsimd.dma_start(
                        a2a_ins[k][:, :, slot, :].rearrange("c p i -> p c i"),
                        outT[:, b, ib, :, :],
                    )
                    if coll is not None:
                        nc.gpsimd.collective_compute(
                            "AllToAll",
                            mybir.AluOpType.bypass,
                            replica_groups=REPLICA_GROUPS,
                            ins=[a2a_ins[coll].opt()],
                            outs=[a2a_outs[coll].opt()],
                        )
                return emit

            coll_of = {}
            for k, blks in enumerate(COLLS):
                for slot, blk in enumerate(blks):
                    coll_of[blk] = (k, slot)

            def attention_block(b, ib, fillers):
                isl = slice(ib * IB, (ib + 1) * IB)
                ptile = pt_pool.tile(
                    [P, JC, HPC, IB], BF16, tag="pt", name="ptile"
                )
                psO = o_psum.tile([P, HPC, IB], FP32, tag="po", name="o_ps")
                last = (b, ib) == COLLS[-1][-1]
                for jc in range(JC):
                    st = st_psum.tile(
                        [P, HPC, IB], FP32, tag="st", name="st_ps"
                    )
                    for h in range(HPC):
                        hsl = slice(h * DH, (h + 1) * DH)
                        nc.tensor.matmul(
                            st[:, h, :],
                            kT[hsl, b, jc * P : (jc + 1) * P],
                            qT[hsl, b, isl],
                            start=True,
                            stop=True,
                        )
                    nc.scalar.activation(
                        ptile[:, jc, :, :],
                        st,
                        mybir.ActivationFunctionType.Exp,
                        scale=SCALE,
                    )
                    pending.append(attnv_closure(b, ib, jc, ptile, psO))
                    while len(pending) > 5:
                        pending.popleft()()
                    if fillers:
                        fillers.pop(0)()
                kend = next(
                    (k for k, blks in enumerate(COLLS) if blks[-1] == (b, ib)),
                    None,
                )
                box = []
                pending.append(norm_a_closure(psO, box))
                pending.append(norm_b_closure(b, ib, box, coll=kend))
                if last:
                    while pending:
                        pending.popleft()()

            def receive(k):
                b0 = 4 * k if k < 2 else 6
                nb = len(COLLS[k])
                nc.sync.dma_start(
                    attnT[:, :, b0 : b0 + nb, :],
                    a2a_outs[k].rearrange("s p q i -> p s q i"),
                )

            # ---- program order ----
            for nb in range(NIB):
                qk_unit(0, xT0, wk_sb, kT, nb)
            for nb in range(NIB):
                qk_unit(0, xT0, wq_sb, qT, nb)

            attention_block(0, 0, [lambda m=m: v_unit(0, xT0, m) for m in range(JC)])
            attention_block(0, 1, [lambda n=n: qk_unit(1, xT1, wk_sb, kT, n) for n in range(NIB)]
                                  + [lambda n=n: qk_unit(1, xT1, wq_sb, qT, n) for n in range(NIB)])
            attention_block(0, 2, [lambda m=m: v_unit(1, xT1, m) for m in range(8)])
            attention_block(0, 3, [lambda m=m: v_unit(1, xT1, m) for m in range(8, JC)])
            attention_block(1, 0, [])
            receive(0)
            attention_block(1, 1, [])
            attention_block(1, 2, [lambda c=c: final_unit(0, c) for c in range(KO)])
            receive(1)
            attention_block(1, 3, [])
            # tail: output projection for collectives 1 and 2
            for cc in range(KO):
                final_unit(1, cc)
            receive(2)
            for cc in range(KO):
                final_unit(2, cc)

    nc.finalize()
    return nc


def _get_nc():
    if "nc" not in _NC_CACHE:
        _NC_CACHE["nc"] = _build()
    return _NC_CACHE["nc"]


def kernel(**inputs) -> np.ndarray:
    import os

    import ml_dtypes

    global LAST_RESULTS

    bf16 = ml_dtypes.bfloat16
    x = np.asarray(inputs["x"], dtype=np.float32)
    W_qkv = np.asarray(inputs["W_qkv"], dtype=np.float32)
    W_out = np.asarray(inputs["W_out"], dtype=np.float32)
    b_out = np.ascontiguousarray(np.asarray(inputs["b_out"], dtype=np.float32))

    x_bf = np.ascontiguousarray(
        x.transpose(0, 2, 1).reshape(B * DIM, N).astype(bf16)
    )
    wo_bf = np.ascontiguousarray(W_out.astype(bf16))
    wqkv_bf = W_qkv.astype(bf16)

    nc = _get_nc()

    in_maps = []
    for c in range(NCORES):
        in_maps.append(
            {
                "x": x_bf,
                "wq": np.ascontiguousarray(
                    wqkv_bf[:, 0 * INNER + c * SH : 0 * INNER + (c + 1) * SH]
                ),
                "wk": np.ascontiguousarray(
                    wqkv_bf[:, 1 * INNER + c * SH : 1 * INNER + (c + 1) * SH]
                ),
                "wv": np.ascontiguousarray(
                    wqkv_bf[:, 2 * INNER + c * SH : 2 * INNER + (c + 1) * SH]
                ),
                "wo": wo_bf,
                "bo": b_out,
            }
        )

    trace = os.environ.get("BASS_KERNEL_TRACE", "0") == "1"
    res = run_bass_kernel_spmd(
        nc, in_maps, core_ids=list(range(NCORES)), trace=trace
    )
    LAST_RESULTS = res

    y = np.empty((B, N, DIM), dtype=np.float32)
    for c in range(NCORES):
        o = res.results[c]["out"]  # [DIM, NCOLL, RPC]
        for k in range(NCOLL):
            b, ib = k // NIB, k % NIB
            r0 = ib * IB + c * RPC
            y[b, r0 : r0 + RPC, :] = o[:, k, :].T
    return y


# revision 22
# speedup vs baseline: 1.0555x; 1.0555x over previous
"""Distributed multi-head attention kernel for 8 TRN2 NeuronCores.

Sharding: 8-way head parallel (2 heads per core), batches looped on-core.

Design (v3):
- attn@V is V-stationary: out^T[d(+denom), i] = v_aug[j, 65].T @ ptile[j, i]
  accumulated over 16 j-chunks with N=512 streaming (vs 1024 tiny N=65
  matmuls in v1 that were LDWEIGHTS-bound at 240us of PE time).
- QK^T dots are ROW-TILED: qT/kT are packed [h*64+d, i] (head 0 in
  partitions 0-63, head 1 in 64-127) and the two heads' dots run
  CONCURRENTLY on the PE's 64-row sub-arrays (verified ~4ns apart on HW).
- The softmax exp stream on the Scalar engine (128 ACTIVATEs x ~1.35us =
  173us) is the roofline wall; every other engine is scheduled around
  keeping it fed.  Engines are FIFO, so PE work is emitted in fine-grained
  slots: each j-chunk slot emits [ST pair][one pending closure][one filler
  unit].  Pending closures carry attn@V pairs 4 slots behind their exp
  (so they never head-of-line block on ACT) and the normalize+staging of
  the previous block; filler units are 8-matmul chunks of the qkv
  projections for batch 1 and the batch-0 output projection.
- Normalize: psO row 64 holds the denominators (ones-column of v_aug).
  psO is copied to SBUF (unnorm + denom rows) to free the single psO
  buffer quickly, then reciprocal -> rank-1 PE broadcast (ones[1,64].T @
  recip) into a q_psum bank -> one DVE multiply per head (SBUF x PSUM).
- Collectives have a ~28us ncfw latency floor regardless of size, so only
  3 AllToAlls: batch 0 (fires at half-time, hidden), blocks (1,0)+(1,1)
  (hidden under the last blocks), (1,2)+(1,3) in the tail.  The gpsimd
  queue carries ONLY staging DMAs + collective triggers (a collective
  trigger blocks the gpsimd engine until completion); receives ride sync.
- The [d, i] output orientation makes receives pure DMA (no transposes);
  the x prologue streams over 3 DMA queues so the first exp fires ~18us in.

The per-core output is the TRANSPOSED final slice [1024, 8, 64] (cols
keyed by (batch, ib, 64-row chunk)); the host transposes during assembly.
"""
from collections import deque

import numpy as np

import concourse.bass as bass
import concourse.mybir as mybir
from concourse import bacc
import concourse.tile as tile
from concourse.bass_utils import run_bass_kernel_spmd

# problem constants (hardcoded; kernel.py must be self-contained)
B, N, DIM = 2, 2048, 1024
H, DH = 16, 64
INNER = H * DH            # 1024
SCALE = DIM ** -0.5       # 1/32  (module scales by dim**-0.5, not dim_head)
NCORES = 8
HPC = H // NCORES         # 2 heads per core
SH = HPC * DH             # 128 inner cols per core
P = 128
KO = DIM // P             # 8 contraction chunks
JC = N // P               # 16 key chunks
IB = 512                  # query block size
NIB = N // IB             # 4 query blocks per batch
NCOLL = B * NIB           # 8 (batch, query-block) output blocks
RPC = IB // NCORES        # 64 rows per core per block
FP32 = mybir.dt.float32
BF16 = mybir.dt.bfloat16

REPLICA_GROUPS = [[0, 1, 2, 3, 4, 5, 6, 7]]
# collective -> list of (b, ib) blocks it carries
COLLS = [
    [(0, 0), (0, 1), (0, 2), (0, 3)],
    [(1, 0), (1, 1), (1, 2), (1, 3)],
]

_NC_CACHE = {}

# set by the last kernel() call when BASS_KERNEL_TRACE=1 (for test.py)
LAST_RESULTS = None


def _build():
    nc = bacc.Bacc(num_devices=NCORES)

    x_ext = nc.declare_dram_parameter("x", [B * DIM, N], BF16, isOutput=False)
    wq_ext = nc.declare_dram_parameter("wq", [DIM, SH], BF16, isOutput=False)
    wk_ext = nc.declare_dram_parameter("wk", [DIM, SH], BF16, isOutput=False)
    wv_ext = nc.declare_dram_parameter("wv", [DIM, SH], BF16, isOutput=False)
    wo_ext = nc.declare_dram_parameter("wo", [DIM, DIM], BF16, isOutput=False)
    bo_ext = nc.declare_dram_parameter("bo", [DIM], FP32, isOutput=False)
    out_ext = nc.declare_dram_parameter(
        "out", [DIM, NCOLL, RPC], FP32, isOutput=True
    )

    with tile.TileContext(nc) as tc:
        with (
            tc.tile_pool(name="consts", bufs=1) as consts,
            tc.tile_pool(name="stage", bufs=2) as stage,
            tc.tile_pool(name="xt_pool", bufs=2) as xt_pool,
            tc.tile_pool(name="pt_pool", bufs=2) as pt_pool,
            tc.tile_pool(name="nrm", bufs=1) as nrm,
            tc.tile_pool(name="unm", bufs=1) as unm,
            tc.tile_pool(name="dram", bufs=1, space="DRAM") as dram,
            tc.tile_pool(name="st_psum", bufs=2, space="PSUM") as st_psum,
            tc.tile_pool(name="o_psum", bufs=1, space="PSUM") as o_psum,
            tc.tile_pool(name="q_psum", bufs=2, space="PSUM") as q_psum,
        ):
            # persistent tensors
            wq_sb = consts.tile([P, KO, SH], BF16)
            wk_sb = consts.tile([P, KO, SH], BF16)
            wv_sb = consts.tile([P, KO, SH], BF16)
            wo_sb = consts.tile([P, KO, DIM], BF16)
            bias_sb = consts.tile([P, KO], FP32)
            ones64 = consts.tile([1, DH], BF16)
            # packed [h*64+d, i]: head h of this core in partitions h*64..
            qT = consts.tile([P, B, N], BF16)
            kT = consts.tile([P, B, N], BF16)
            v_aug = consts.tile([P, B, JC, HPC, DH + 1], BF16)
            # normalized transposed attention output [h*64+d, b, ib, c, i]
            outT = consts.tile([P, B, NIB, NCORES, RPC], BF16)
            # received: [inner mod 128, src core(=inner/128), block, row]
            attnT = consts.tile([P, KO, NCOLL, RPC], BF16)

            a2a_ins = [
                dram.tile([NCORES, P, len(blks), RPC], BF16, name=f"a2a_in{k}")
                for k, blks in enumerate(COLLS)
            ]
            a2a_outs = [
                dram.tile([NCORES, P, len(blks), RPC], BF16, name=f"a2a_out{k}")
                for k, blks in enumerate(COLLS)
            ]

            # preload the exp table right away (dummy activation on ones64)
            nc.vector.memset(ones64, 1.0)
            warm = consts.tile([1, 2], FP32)
            nc.scalar.activation(
                warm, ones64[:, 0:2], mybir.ActivationFunctionType.Exp
            )
            nc.vector.memset(v_aug[:, :, :, :, DH : DH + 1], 1.0)

            # ---- input DMAs spread over 3 queues (sync/scalar/gpsimd) ----
            def load_x_nb(b, nb, xT, eng):
                for ko in range(KO):
                    eng.dma_start(
                        xT[:, ko, nb * IB : (nb + 1) * IB],
                        x_ext[
                            b * DIM + ko * P : b * DIM + (ko + 1) * P,
                            nb * IB : (nb + 1) * IB,
                        ],
                    )

            def load_w(w_ext, w_sb):
                nc.scalar.dma_start(
                    w_sb, w_ext.rearrange("(ko kp) c -> kp ko c", kp=P)
                )

            xT0 = xt_pool.tile([P, KO, N], BF16, tag="xT", name="xT0")
            xT1 = xt_pool.tile([P, KO, N], BF16, tag="xT", name="xT1")
            load_w(wk_ext, wk_sb)
            load_w(wq_ext, wq_sb)
            load_x_nb(0, 0, xT0, nc.sync)
            load_x_nb(0, 1, xT0, nc.scalar)
            load_x_nb(0, 2, xT0, nc.gpsimd)
            load_x_nb(0, 3, xT0, nc.sync)
            load_w(wv_ext, wv_sb)
            load_x_nb(1, 0, xT1, nc.sync)
            load_x_nb(1, 1, xT1, nc.scalar)
            load_x_nb(1, 2, xT1, nc.gpsimd)
            load_x_nb(1, 3, xT1, nc.sync)
            load_w(wo_ext, wo_sb)
            nc.scalar.dma_start(
                bias_sb, bo_ext.rearrange("(co cp) -> cp co", cp=P)
            )

            # warm the PE (HAM un-throttle) while x streams in: ~14 junk
            # matmuls with no input dependencies
            junk = consts.tile([P, IB], BF16)
            nc.vector.memset(junk[:, 0:1], 0.0)
            nc.vector.memset(junk, 0.125)
            for _ in range(14):
                ps_w = q_psum.tile([P, IB], FP32, tag="qk", name="warm_ps")
                nc.tensor.matmul(
                    ps_w, junk[:, 0:P], junk, start=True, stop=True
                )

            # ---- filler units (each ~8 matmuls + a DVE drain) ----
            def qk_unit(b, xT, w_sb, dstT, nb):
                ps = q_psum.tile([P, IB], FP32, tag="qk", name="qk_ps")
                for ko in range(KO):
                    nc.tensor.matmul(
                        ps,
                        w_sb[:, ko, :],
                        xT[:, ko, nb * IB : (nb + 1) * IB],
                        start=(ko == 0),
                        stop=(ko == KO - 1),
                    )
                nc.vector.tensor_copy(dstT[:, b, nb * IB : (nb + 1) * IB], ps)

            def v_unit(b, xT, mt):
                ps_v = q_psum.tile([P, SH], FP32, tag="qk", name="v_ps")
                for ko in range(KO):
                    nc.tensor.matmul(
                        ps_v,
                        xT[:, ko, mt * P : (mt + 1) * P],
                        wv_sb[:, ko, :],
                        start=(ko == 0),
                        stop=(ko == KO - 1),
                    )
                nc.vector.tensor_copy(
                    v_aug[:, b, mt, :, 0:DH],
                    ps_v.rearrange("p (h d) -> p h d", d=DH),
                )

            def final_unit(k, cc):
                """Output projection for collective k's blocks, one 128-col
                chunk of DIM; bias add + result DMA (sync queue)."""
                b0 = 4 * k if k < 2 else 6
                nb = len(COLLS[k])
                csl = slice(b0, b0 + nb)
                ps_f4 = q_psum.tile([P, NIB, RPC], FP32, tag="qk", name="f_ps")
                ps_f = ps_f4[:, 0:nb, :]
                for ko in range(KO):
                    nc.tensor.matmul(
                        ps_f,
                        wo_sb[:, ko, cc * P : (cc + 1) * P],
                        attnT[:, ko, csl, :],
                        start=(ko == 0),
                        stop=(ko == KO - 1),
                    )
                of4 = stage.tile([P, NIB, RPC], FP32, tag="of", name="of")
                of = of4[:, 0:nb, :]
                nc.vector.tensor_scalar_add(of, ps_f, bias_sb[:, cc : cc + 1])
                nc.sync.dma_start(out_ext[cc * P : (cc + 1) * P, csl, :], of)

            # ---- attention blocks with slot-based emission ----
            pending = deque()  # closures: attn@V pairs (trailing) + normalize

            def attnv_closure(b, ib, jc, ptile, psO):
                def emit():
                    for h in range(HPC):
                        nc.tensor.matmul(
                            psO[0 : DH + 1, h, :],
                            v_aug[:, b, jc, h, :],
                            ptile[:, jc, h, :],
                            start=(jc == 0),
                            stop=(jc == JC - 1),
                        )
                return emit

            def norm_a_closure(psO, box):
                def emit():
                    # free psO fast: copy unnormalized rows + denominators out
                    unnorm = unm.tile([DH, HPC, IB], BF16, tag="un", name="un")
                    dnm = nrm.tile([1, HPC, IB], FP32, tag="dn", name="dn")
                    nc.vector.tensor_copy(unnorm, psO[0:DH, :, :])
                    nc.vector.tensor_copy(dnm, psO[DH : DH + 1, :, :])
                    recip = nrm.tile([1, HPC, IB], FP32, tag="rc", name="rc")
                    nc.vector.reciprocal(recip, dnm)
                    rc_bf = nrm.tile([1, HPC, IB], BF16, tag="rcb", name="rcb")
                    nc.vector.tensor_copy(rc_bf, recip)
                    box.append((unnorm, rc_bf))
                return emit

            def norm_b_closure(b, ib, box, coll=None):
                def emit():
                    unnorm, rc_bf = box.pop()
                    for h in range(HPC):
                        psB = q_psum.tile([P, IB], FP32, tag="qk", name="b_ps")
                        nc.tensor.matmul(
                            psB[0:DH, :],
                            ones64,
                            rc_bf[:, h, :],
                            start=True,
                            stop=True,
                        )
                        nc.vector.tensor_tensor(
                            outT[h * DH : (h + 1) * DH, b, ib, :, :],
                            unnorm[:, h, :].rearrange(
                                "d (c i) -> d c i", c=NCORES
                            ),
                            psB[0:DH, :].rearrange("d (c i) -> d c i", c=NCORES),
                            mybir.AluOpType.mult,
                        )
                    # stage this block into its collective's buffer
                    k, slot = coll_of[(b, ib)]
                    nc.gpsimd.dma_start(
                        a2a_ins[k][:, :, slot, :].rearrange("c p i -> p c i"),
                        outT[:, b, ib, :, :],
                    )
                    if coll is not None:
                        nc.gpsimd.collective_compute(
                            "AllToAll",
                            mybir.AluOpType.bypass,
                            replica_groups=REPLICA_GROUPS,
                            ins=[a2a_ins[coll].opt()],
                            outs=[a2a_outs[coll].opt()],
                        )
                return emit

            coll_of = {}
            for k, blks in enumerate(COLLS):
                for slot, blk in enumerate(blks):
                    coll_of[blk] = (k, slot)

            def attention_block(b, ib, fillers, double_slots=0):
                isl = slice(ib * IB, (ib + 1) * IB)
                ptile = pt_pool.tile(
                    [P, JC, HPC, IB], BF16, tag="pt", name="ptile"
                )
                psO = o_psum.tile([P, HPC, IB], FP32, tag="po", name="o_ps")
                last = (b, ib) == COLLS[-1][-1]
                for jc in range(JC):
                    st = st_psum.tile(
                        [P, HPC, IB], FP32, tag="st", name="st_ps"
                    )
                    for h in range(HPC):
                        hsl = slice(h * DH, (h + 1) * DH)
                        nc.tensor.matmul(
                            st[:, h, :],
                            kT[hsl, b, jc * P : (jc + 1) * P],
                            qT[hsl, b, isl],
                            start=True,
                            stop=True,
                        )
                    nc.scalar.activation(
                        ptile[:, jc, :, :],
                        st,
                        mybir.ActivationFunctionType.Exp,
                        scale=SCALE,
                    )
                    pending.append(attnv_closure(b, ib, jc, ptile, psO))
                    while len(pending) > 5:
                        pending.popleft()()
                    for _ in range(2 if jc < double_slots else 1):
                        if fillers:
                            fillers.pop(0)()
                kend = next(
                    (k for k, blks in enumerate(COLLS) if blks[-1] == (b, ib)),
                    None,
                )
                box = []
                pending.append(norm_a_closure(psO, box))
                pending.append(norm_b_closure(b, ib, box, coll=kend))
                if last:
                    while pending:
                        pending.popleft()()

            def receive(k):
                b0 = 4 * k if k < 2 else 6
                nb = len(COLLS[k])
                nc.sync.dma_start(
                    attnT[:, :, b0 : b0 + nb, :],
                    a2a_outs[k].rearrange("s p q i -> p s q i"),
                )

            # ---- program order ----
            # only k (all blocks) + q (block 0) gate the first QK^T dots;
            # the rest of the projections ride the filler queue
            for nb in range(NIB):
                qk_unit(0, xT0, wk_sb, kT, nb)
            qk_unit(0, xT0, wq_sb, qT, 0)

            fillers = (
                [lambda n=n: qk_unit(0, xT0, wq_sb, qT, n) for n in range(1, NIB)]
                + [lambda m=m: v_unit(0, xT0, m) for m in range(JC)]
                + [lambda n=n: qk_unit(1, xT1, wk_sb, kT, n) for n in range(NIB)]
                + [lambda n=n: qk_unit(1, xT1, wq_sb, qT, n) for n in range(NIB)]
                + [lambda m=m: v_unit(1, xT1, m) for m in range(JC)]
            )
            for b in range(B):
                for ib in range(NIB):
                    attention_block(
                        b, ib, fillers, double_slots=3 if (b, ib) == (0, 0) else 0
                    )
                    if (b, ib) == (1, 0):
                        receive(0)
            # tail: batch-0 projection hides under the batch-1 collective
            for cc in range(KO):
                final_unit(0, cc)
            receive(1)
            for cc in range(KO):
                final_unit(1, cc)

    nc.finalize()
    return nc


def _get_nc():
    if "nc" not in _NC_CACHE:
        _NC_CACHE["nc"] = _build()
    return _NC_CACHE["nc"]


def kernel(**inputs) -> np.ndarray:
    import os

    import ml_dtypes

    global LAST_RESULTS

    bf16 = ml_dtypes.bfloat16
    x = np.asarray(inputs["x"], dtype=np.float32)
    W_qkv = np.asarray(inputs["W_qkv"], dtype=np.float32)
    W_out = np.asarray(inputs["W_out"], dtype=np.float32)
    b_out = np.ascontiguousarray(np.asarray(inputs["b_out"], dtype=np.float32))

    x_bf = np.ascontiguousarray(
        x.transpose(0, 2, 1).reshape(B * DIM, N).astype(bf16)
    )
    wo_bf = np.ascontiguousarray(W_out.astype(bf16))
    wqkv_bf = W_qkv.astype(bf16)

    nc = _get_nc()

    in_maps = []
    for c in range(NCORES):
        in_maps.append(
            {
                "x": x_bf,
                "wq": np.ascontiguousarray(
                    wqkv_bf[:, 0 * INNER + c * SH : 0 * INNER + (c + 1) * SH]
                ),
                "wk": np.ascontiguousarray(
                    wqkv_bf[:, 1 * INNER + c * SH : 1 * INNER + (c + 1) * SH]
                ),
                "wv": np.ascontiguousarray(
                    wqkv_bf[:, 2 * INNER + c * SH : 2 * INNER + (c + 1) * SH]
                ),
                "wo": wo_bf,
                "bo": b_out,
            }
        )

    trace = os.environ.get("BASS_KERNEL_TRACE", "0") == "1"
    res = run_bass_kernel_spmd(
        nc, in_maps, core_ids=list(range(NCORES)), trace=trace
    )
    LAST_RESULTS = res

    y = np.empty((B, N, DIM), dtype=np.float32)
    for c in range(NCORES):
        o = res.results[c]["out"]  # [DIM, NCOLL, RPC]
        for k in range(NCOLL):
            b, ib = k // NIB, k % NIB
            r0 = ib * IB + c * RPC
            y[b, r0 : r0 + RPC, :] = o[:, k, :].T
    return y


# revision 28
# speedup vs baseline: 1.2438x; 1.1784x over previous
"""Distributed multi-head attention kernel for 8 TRN2 NeuronCores.

Sharding: 8-way head parallel (2 heads per core), batches looped on-core.
Each core: QKV projection for its 2 heads over both batches, per-head
attention (softmax without max-subtraction — logits are small; denominators
come from a ones-column appended to V so they fall out of the attn@V
matmul), then per-head AllToAlls across all 8 cores exchange row-blocks
for head-blocks: block s = (batch s//4, rows-block s%4).  Core c ends up
with all 1024 inner dims for (batch c//4, rows [(c%4)*512, ...)) and runs
the full output projection + bias on that slice.  The head-0 A2A fires at
attention half-time and overlaps head-1 compute; the exchange moves 1MB
of bf16 per core instead of all-reducing 8.4MB of fp32.

x and the weights are cast to bf16 on the host (bf16 is the compute
precision anyway) and x additionally arrives pre-transposed, so x^T
streams straight into SBUF with fully contiguous DMAs — no on-chip
staging casts and no x transposes.  The remaining transposes (A2A
receive) run on the PE array (XBAR transposes proved slow for narrow
blocks and corrupt data when issued on two HWDGE queues concurrently).

Program order interleaves qkv of batch 1 with attention of (head 0,
batch 0) so the Scalar engine (exp — the second-busiest engine) starts
~45us in instead of after all projections.

The per-core output is the TRANSPOSED final slice [1024, 512] (PSUM-major
writes stay contiguous); the host transposes during assembly.
"""
import numpy as np

import concourse.bass as bass
import concourse.mybir as mybir
from concourse import bacc
import concourse.tile as tile
from concourse.bass_utils import run_bass_kernel_spmd
from concourse.masks import make_identity

# problem constants (hardcoded; kernel.py must be self-contained)
B, N, DIM = 2, 2048, 1024
H, DH = 16, 64
INNER = H * DH            # 1024
SCALE = DIM ** -0.5       # 1/32  (module scales by dim**-0.5, not dim_head)
NCORES = 8
HPC = H // NCORES         # 2 heads per core
SH = HPC * DH             # 128 inner cols per core
ROWS = N // 4             # 512 output rows per core
P = 128
KO = DIM // P             # 8 contraction chunks
JC = N // P               # 16 row chunks
IB = 512                  # query block size
NIB = N // IB             # 4 query blocks
ISUB = IB // P            # 4
FP32 = mybir.dt.float32
BF16 = mybir.dt.bfloat16

REPLICA_GROUPS = [[0, 1, 2, 3, 4, 5, 6, 7]]

_NC_CACHE = {}

# set by the last kernel() call when BASS_KERNEL_TRACE=1 (for test.py)
LAST_RESULTS = None


def _build():
    nc = bacc.Bacc(num_devices=NCORES)

    x_ext = nc.declare_dram_parameter("x", [B * DIM, N], BF16, isOutput=False)
    wq_ext = nc.declare_dram_parameter("wq", [DIM, SH], BF16, isOutput=False)
    wk_ext = nc.declare_dram_parameter("wk", [DIM, SH], BF16, isOutput=False)
    wv_ext = nc.declare_dram_parameter("wv", [DIM, SH], BF16, isOutput=False)
    wo_ext = nc.declare_dram_parameter("wo", [DIM, DIM], BF16, isOutput=False)
    bo_ext = nc.declare_dram_parameter("bo", [DIM], FP32, isOutput=False)
    out_ext = nc.declare_dram_parameter("out", [DIM, ROWS], FP32, isOutput=True)

    with tile.TileContext(nc) as tc:
        with (
            tc.tile_pool(name="consts", bufs=1) as consts,
            tc.tile_pool(name="stage", bufs=3) as stage,
            tc.tile_pool(name="xt_pool", bufs=2) as xt_pool,
            tc.tile_pool(name="pt_pool", bufs=2) as pt_pool,
            tc.tile_pool(name="nrm", bufs=8) as nrm,
            tc.tile_pool(name="rst_pool", bufs=8) as rst_pool,
            tc.tile_pool(name="dram", bufs=1, space="DRAM") as dram,
            tc.tile_pool(name="st_psum", bufs=3, space="PSUM") as st_psum,
            tc.tile_pool(name="o_psum", bufs=2, space="PSUM") as o_psum,
        ):
            ident_bf = consts.tile([P, P], BF16)

            # persistent tensors
            wq_sb = consts.tile([P, KO, SH], BF16)
            wk_sb = consts.tile([P, KO, SH], BF16)
            wv_sb = consts.tile([P, KO, SH], BF16)
            wo_sb = consts.tile([P, KO, DIM], BF16)
            bias_sb = consts.tile([P, KO], FP32)
            qT = consts.tile([P, B, HPC, N], BF16)   # [d(+zero pad), b, h, i]
            kT = consts.tile([P, B, HPC, N], BF16)
            v_aug = consts.tile([P, B, JC, HPC, DH + 1], BF16)
            out_rows = consts.tile([P, B, JC, SH], BF16)
            attnT = xt_pool.tile([P, KO, N], BF16, tag="xT", name="attnT")[
                :, :, :ROWS
            ]

            a2a_in0 = dram.tile([NCORES, P, NIB, DH], BF16, name="a2a_in0")
            a2a_in1 = dram.tile([NCORES, P, NIB, DH], BF16, name="a2a_in1")
            a2a_out0 = dram.tile([NCORES, P, NIB, DH], BF16, name="a2a_out0")
            a2a_out1 = dram.tile([NCORES, P, NIB, DH], BF16, name="a2a_out1")
            a2a_ins = [a2a_in0, a2a_in1]
            a2a_outs = [a2a_out0, a2a_out1]

            nc.gpsimd.memset(qT[DH:P, :, :, :], 0.0)
            nc.gpsimd.memset(kT[DH:P, :, :, :], 0.0)
            nc.vector.memset(v_aug[:, :, :, :, DH : DH + 1], 1.0)
            make_identity(nc, ident_bf)

            # junk operand for PE warm-up / warm-keeper matmuls (HAM needs
            # ~3.4us of sustained PE activity to un-throttle 1.2->2.4 GHz)
            junk = consts.tile([P, IB], BF16)
            nc.vector.memset(junk, 0.125)

            def pe_warm(n):
                for _ in range(n):
                    psw = st_psum.tile([P, 2, IB], FP32, tag="st", name="warm")
                    nc.tensor.matmul(
                        psw[:, 0, :], junk[:, 0:P], junk, start=True, stop=True
                    )

            def load_xT(b):
                """x[b]^T arrives pre-transposed from the host: straight
                contiguous loads spread over 3 DMA queues."""
                xT = xt_pool.tile([P, KO, N], BF16, tag="xT", name="xT")
                engs = [nc.sync, nc.scalar, nc.gpsimd, nc.sync]
                for nb in range(NIB):
                    for ko in range(KO):
                        engs[nb].dma_start(
                            xT[:, ko, nb * IB : (nb + 1) * IB],
                            x_ext[
                                b * DIM + ko * P : b * DIM + (ko + 1) * P,
                                nb * IB : (nb + 1) * IB,
                            ],
                        )
                return xT

            def qk_proj(b, xT):
                # k first (attention's dots consume kT earliest), then q
                for w_sb, dstT in ((wk_sb, kT), (wq_sb, qT)):
                    for nb in range(NIB):
                        ps2 = st_psum.tile(
                            [P, 2, IB], FP32, tag="st", name="qk_ps"
                        )
                        ps = ps2[:, 0, :]
                        for ko in range(KO):
                            nc.tensor.matmul(
                                ps,
                                w_sb[:, ko, :],
                                xT[:, ko, nb * IB : (nb + 1) * IB],
                                start=(ko == 0),
                                stop=(ko == KO - 1),
                            )
                        for h in range(HPC):
                            nc.vector.tensor_copy(
                                dstT[0:DH, b, h, nb * IB : (nb + 1) * IB],
                                ps[h * DH : (h + 1) * DH, :],
                            )

            def load_qkv_weights():
                for w_ext, w_sb in (
                    (wk_ext, wk_sb),
                    (wq_ext, wq_sb),
                    (wv_ext, wv_sb),
                ):
                    nc.scalar.dma_start(
                        w_sb, w_ext.rearrange("(ko kp) c -> kp ko c", kp=P)
                    )

            def load_out_weights():
                nc.scalar.dma_start(
                    wo_sb, wo_ext.rearrange("(ko kp) c -> kp ko c", kp=P)
                )
                nc.scalar.dma_start(
                    bias_sb, bo_ext.rearrange("(co cp) -> cp co", cp=P)
                )

            def v_proj(b, xT):
                for mt in range(JC):
                    psv2 = st_psum.tile([P, 2, IB], FP32, tag="st", name="v_ps")
                    ps_v = psv2[:, 0, :SH]
                    for ko in range(KO):
                        nc.tensor.matmul(
                            ps_v,
                            xT[:, ko, mt * P : (mt + 1) * P],
                            wv_sb[:, ko, :],
                            start=(ko == 0),
                            stop=(ko == KO - 1),
                        )
                    nc.vector.tensor_copy(
                        v_aug[:, b, mt, :, 0:DH],
                        ps_v.rearrange("p (h d) -> p h d", d=DH),
                    )

            def attention(h, b):
                """ST = k@q.T per j-chunk, exp on ACT, attn@V with the
                denominator in column DH.  All 4 i-sub accumulations of a
                block share one PSUM bank so the 'po' slots rotate once
                per block, not once per i-sub (normalizes are emitted
                after all 64 matmuls so the bank tracker doesn't
                interleave PE writes with DVE reads)."""
                po = h * DH
                for ib in range(NIB):
                    ptile = pt_pool.tile([P, JC, IB], BF16, tag="pt", name="ptile")
                    for jg in range(JC // 2):
                        ps_st = st_psum.tile(
                            [P, 2, IB], FP32, tag="st", name="st_ps"
                        )
                        for u in range(2):
                            jc = jg * 2 + u
                            nc.tensor.matmul(
                                ps_st[:, u, :],
                                kT[:, b, h, jc * P : (jc + 1) * P],
                                qT[:, b, h, ib * IB : (ib + 1) * IB],
                                start=True,
                                stop=True,
                            )
                        nc.scalar.activation(
                            ptile[:, jg * 2 : (jg + 1) * 2, :],
                            ps_st,
                            mybir.ActivationFunctionType.Exp,
                            scale=SCALE,
                        )
                    ps_o4 = o_psum.tile(
                        [P, ISUB, DH + 1], FP32, tag="po", name="o_ps"
                    )
                    for isub in range(ISUB):
                        for jc in range(JC):
                            nc.tensor.matmul(
                                ps_o4[:, isub, :],
                                ptile[:, jc, isub * P : (isub + 1) * P],
                                v_aug[:, b, jc, h, :],
                                start=(jc == 0),
                                stop=(jc == JC - 1),
                            )
                    for isub in range(ISUB):
                        ic = ib * ISUB + isub
                        recip = nrm.tile([P, 1], FP32, tag="recip", name="recip")
                        nc.vector.reciprocal(recip, ps_o4[:, isub, DH : DH + 1])
                        nc.vector.tensor_scalar_mul(
                            out_rows[:, b, ic, po : po + DH],
                            ps_o4[:, isub, 0:DH],
                            recip,
                        )
                    # block (h, b, ib) complete -> stage its A2A input
                    s = b * NIB + ib
                    nc.sync.dma_start(
                        a2a_ins[h][s],
                        out_rows[
                            :, b, ib * ISUB : (ib + 1) * ISUB, po : po + DH
                        ],
                    )

            def a2a_exchange(h):
                nc.gpsimd.collective_compute(
                    "AllToAll",
                    mybir.AluOpType.bypass,
                    replica_groups=REPLICA_GROUPS,
                    ins=[a2a_ins[h].opt()],
                    outs=[a2a_outs[h].opt()],
                )

            def receive(h):
                po = h * DH
                rstages = []
                for i in range(NCORES):
                    rstage = rst_pool.tile(
                        [P, NIB, DH], BF16, tag="rstage", name="rstage"
                    )
                    nc.scalar.dma_start(rstage, a2a_outs[h][i])
                    rstages.append(rstage)
                for i in range(NCORES):
                    rps = st_psum.tile([DH, NIB, P], BF16, tag="st", name="r_ps")
                    for q in range(NIB):
                        nc.tensor.transpose(
                            rps[:, q, :], rstages[i][:, q, :], ident_bf
                        )
                    nc.vector.tensor_copy(attnT[po : po + DH, i, :], rps)

            def final_projection():
                for cc in range(KO):
                    psf2 = st_psum.tile([P, 2, IB], FP32, tag="st", name="f_ps")
                    ps_f = psf2[:, 0, :ROWS]
                    for ko in range(KO):
                        nc.tensor.matmul(
                            ps_f,
                            wo_sb[:, ko, cc * P : (cc + 1) * P],
                            attnT[:, ko, :],
                            start=(ko == 0),
                            stop=(ko == KO - 1),
                        )
                    of = stage.tile([P, ROWS], FP32, tag="of", name="of")
                    nc.vector.tensor_scalar_add(of, ps_f, bias_sb[:, cc : cc + 1])
                    nc.sync.dma_start(out_ext[cc * P : (cc + 1) * P, :], of)

            # ---- program order chosen so exp starts early and the PE
            # always has lower-priority projection work to fill gaps ----
            load_qkv_weights()
            xT0 = load_xT(0)
            pe_warm(10)
            qk_proj(0, xT0)
            v_proj(0, xT0)
            attention(0, 0)
            xT1 = load_xT(1)
            load_out_weights()
            qk_proj(1, xT1)
            v_proj(1, xT1)
            attention(0, 1)
            a2a_exchange(0)        # flies under head-1 attention
            attention(1, 0)
            attention(1, 1)
            receive(0)             # PE work during the head-1 A2A flight
            a2a_exchange(1)
            pe_warm(110)           # keep HAM at 8/8 across the A2A-1 flight
            receive(1)
            final_projection()

    nc.finalize()
    return nc


def _get_nc():
    if "nc" not in _NC_CACHE:
        _NC_CACHE["nc"] = _build()
    return _NC_CACHE["nc"]


def kernel(**inputs) -> np.ndarray:
    import os

    import ml_dtypes

    global LAST_RESULTS

    bf16 = ml_dtypes.bfloat16
    x = np.asarray(inputs["x"], dtype=np.float32)
    W_qkv = np.asarray(inputs["W_qkv"], dtype=np.float32)
    W_out = np.asarray(inputs["W_out"], dtype=np.float32)
    b_out = np.ascontiguousarray(np.asarray(inputs["b_out"], dtype=np.float32))

    x_bf = np.ascontiguousarray(
        x.transpose(0, 2, 1).reshape(B * DIM, N).astype(bf16)
    )
    wo_bf = np.ascontiguousarray(W_out.astype(bf16))
    wqkv_bf = W_qkv.astype(bf16)

    nc = _get_nc()

    in_maps = []
    for c in range(NCORES):
        in_maps.append(
            {
                "x": x_bf,
                "wq": np.ascontiguousarray(
                    wqkv_bf[:, 0 * INNER + c * SH : 0 * INNER + (c + 1) * SH]
                ),
                "wk": np.ascontiguousarray(
                    wqkv_bf[:, 1 * INNER + c * SH : 1 * INNER + (c + 1) * SH]
                ),
                "wv": np.ascontiguousarray(
                    wqkv_bf[:, 2 * INNER + c * SH : 2 * INNER + (c + 1) * SH]
                ),
                "wo": wo_bf,
                "bo": b_out,
            }
        )

    trace = os.environ.get("BASS_KERNEL_TRACE", "0") == "1"
    res = run_bass_kernel_spmd(
        nc, in_maps, core_ids=list(range(NCORES)), trace=trace
    )
    LAST_RESULTS = res

    y = np.empty((B, N, DIM), dtype=np.float32)
    for c in range(NCORES):
        b, r = c // 4, c % 4
        y[b, r * ROWS : (r + 1) * ROWS, :] = res.results[c]["out"].T
    return y



# revision 35
# speedup vs baseline: 1.3893x; 1.1170x over previous
"""Distributed multi-head attention kernel for 8 TRN2 NeuronCores.

Sharding: 8-way head parallel (2 heads per core), batches looped on-core.
Each core: QKV projection for its 2 heads over both batches, per-head
attention (softmax without max-subtraction — logits are small; denominators
come from a ones-column appended to V so they fall out of the attn@V
matmul), then per-head AllToAlls across all 8 cores exchange row-blocks
for head-blocks: block s = (batch s//4, rows-block s%4).  Core c ends up
with all 1024 inner dims for (batch c//4, rows [(c%4)*512, ...)) and runs
the full output projection + bias on that slice.  The head-0 A2A fires at
attention half-time and overlaps head-1 compute; the exchange moves 1MB
of bf16 per core instead of all-reducing 8.4MB of fp32.

x and the weights are cast to bf16 on the host (bf16 is the compute
precision anyway) and x additionally arrives pre-transposed, so x^T
streams straight into SBUF with fully contiguous DMAs — no on-chip
staging casts and no x transposes.  The remaining transposes (A2A
receive) run on the PE array (XBAR transposes proved slow for narrow
blocks and corrupt data when issued on two HWDGE queues concurrently).

Program order interleaves qkv of batch 1 with attention of (head 0,
batch 0) so the Scalar engine (exp — the second-busiest engine) starts
~45us in instead of after all projections.

The per-core output is the TRANSPOSED final slice [1024, 512] (PSUM-major
writes stay contiguous); the host transposes during assembly.
"""
import numpy as np

import concourse.bass as bass
import concourse.mybir as mybir
from concourse import bacc
import concourse.tile as tile
from concourse.bass_utils import run_bass_kernel_spmd
from concourse.masks import make_identity

# problem constants (hardcoded; kernel.py must be self-contained)
B, N, DIM = 2, 2048, 1024
H, DH = 16, 64
INNER = H * DH            # 1024
SCALE = DIM ** -0.5       # 1/32  (module scales by dim**-0.5, not dim_head)
NCORES = 8
HPC = H // NCORES         # 2 heads per core
SH = HPC * DH             # 128 inner cols per core
ROWS = N // 4             # 512 output rows per core
P = 128
KO = DIM // P             # 8 contraction chunks
JC = N // P               # 16 row chunks
IB = 512                  # query block size
NIB = N // IB             # 4 query blocks
ISUB = IB // P            # 4
FP32 = mybir.dt.float32
BF16 = mybir.dt.bfloat16

REPLICA_GROUPS = [[0, 1, 2, 3, 4, 5, 6, 7]]

_NC_CACHE = {}

# set by the last kernel() call when BASS_KERNEL_TRACE=1 (for test.py)
LAST_RESULTS = None


def _build():
    nc = bacc.Bacc(num_devices=NCORES)

    x_ext = nc.declare_dram_parameter("x", [B * DIM, N], BF16, isOutput=False)
    wq_ext = nc.declare_dram_parameter("wq", [DIM, SH], BF16, isOutput=False)
    wk_ext = nc.declare_dram_parameter("wk", [DIM, SH], BF16, isOutput=False)
    wv_ext = nc.declare_dram_parameter("wv", [DIM, SH], BF16, isOutput=False)
    wo_ext = nc.declare_dram_parameter("wo", [DIM, DIM], BF16, isOutput=False)
    bo_ext = nc.declare_dram_parameter("bo", [DIM], FP32, isOutput=False)
    out_ext = nc.declare_dram_parameter("out", [DIM, ROWS], FP32, isOutput=True)

    with tile.TileContext(nc) as tc:
        with (
            tc.tile_pool(name="consts", bufs=1) as consts,
            tc.tile_pool(name="stage", bufs=3) as stage,
            tc.tile_pool(name="xt_pool", bufs=2) as xt_pool,
            tc.tile_pool(name="pt_pool", bufs=2) as pt_pool,
            tc.tile_pool(name="nrm", bufs=8) as nrm,
            tc.tile_pool(name="rst_pool", bufs=8) as rst_pool,
            tc.tile_pool(name="dram", bufs=1, space="DRAM") as dram,
            tc.tile_pool(name="st_psum", bufs=3, space="PSUM") as st_psum,
            tc.tile_pool(name="o_psum", bufs=2, space="PSUM") as o_psum,
        ):
            ident_bf = consts.tile([P, P], BF16)

            # persistent tensors
            wq_sb = consts.tile([P, KO, SH], BF16)
            wk_sb = consts.tile([P, KO, SH], BF16)
            wv_sb = consts.tile([P, KO, SH], BF16)
            wo_sb = consts.tile([P, KO, DIM], BF16)
            bias_sb = consts.tile([P, KO], FP32)
            qT = consts.tile([P, B, HPC, N], BF16)   # [d(+zero pad), b, h, i]
            kT = consts.tile([P, B, HPC, N], BF16)
            v_aug = consts.tile([P, B, JC, HPC, DH + 1], BF16)
            out_rows = consts.tile([P, B, JC, SH], BF16)
            attnT = xt_pool.tile([P, KO, N], BF16, tag="xT", name="attnT")[
                :, :, :ROWS
            ]

            a2a_in0 = dram.tile([NCORES, P, NIB, DH], BF16, name="a2a_in0")
            a2a_in1 = dram.tile([NCORES, P, NIB, DH], BF16, name="a2a_in1")
            a2a_out0 = dram.tile([NCORES, P, NIB, DH], BF16, name="a2a_out0")
            a2a_out1 = dram.tile([NCORES, P, NIB, DH], BF16, name="a2a_out1")
            a2a_ins = [a2a_in0, a2a_in1]
            a2a_outs = [a2a_out0, a2a_out1]

            nc.gpsimd.memset(qT[DH:P, :, :, :], 0.0)
            nc.gpsimd.memset(kT[DH:P, :, :, :], 0.0)
            nc.vector.memset(v_aug[:, :, :, :, DH : DH + 1], 1.0)
            make_identity(nc, ident_bf)

            # junk operand for PE warm-up / warm-keeper matmuls (HAM needs
            # ~3.4us of sustained PE activity to un-throttle 1.2->2.4 GHz)
            junk = consts.tile([P, IB], BF16)
            nc.vector.memset(junk, 0.125)

            def pe_warm(n):
                for _ in range(n):
                    psw = st_psum.tile([P, 2, IB], FP32, tag="st", name="warm")
                    nc.tensor.matmul(
                        psw[:, 0, :], junk[:, 0:P], junk, start=True, stop=True
                    )

            def load_xT(b):
                """x[b]^T arrives pre-transposed from the host: straight
                contiguous loads spread over 3 DMA queues."""
                xT = xt_pool.tile([P, KO, N], BF16, tag="xT", name="xT")
                engs = [nc.sync, nc.scalar, nc.gpsimd, nc.sync]
                for nb in range(NIB):
                    for ko in range(KO):
                        engs[nb].dma_start(
                            xT[:, ko, nb * IB : (nb + 1) * IB],
                            x_ext[
                                b * DIM + ko * P : b * DIM + (ko + 1) * P,
                                nb * IB : (nb + 1) * IB,
                            ],
                        )
                return xT

            def qk_chunk(b, xT, w_sb, dstT, nbs):
                for nb in nbs:
                    ps2 = st_psum.tile([P, 2, IB], FP32, tag="st", name="qk_ps")
                    ps = ps2[:, 0, :]
                    for ko in range(KO):
                        nc.tensor.matmul(
                            ps,
                            w_sb[:, ko, :],
                            xT[:, ko, nb * IB : (nb + 1) * IB],
                            start=(ko == 0),
                            stop=(ko == KO - 1),
                        )
                    for h in range(HPC):
                        nc.vector.tensor_copy(
                            dstT[0:DH, b, h, nb * IB : (nb + 1) * IB],
                            ps[h * DH : (h + 1) * DH, :],
                        )

            def qk_proj(b, xT):
                # k first (attention's dots consume kT earliest), then q
                qk_chunk(b, xT, wk_sb, kT, range(NIB))
                qk_chunk(b, xT, wq_sb, qT, range(NIB))

            def load_qkv_weights():
                for w_ext, w_sb in (
                    (wk_ext, wk_sb),
                    (wq_ext, wq_sb),
                    (wv_ext, wv_sb),
                ):
                    nc.scalar.dma_start(
                        w_sb, w_ext.rearrange("(ko kp) c -> kp ko c", kp=P)
                    )

            def load_out_weights():
                nc.scalar.dma_start(
                    wo_sb, wo_ext.rearrange("(ko kp) c -> kp ko c", kp=P)
                )
                nc.scalar.dma_start(
                    bias_sb, bo_ext.rearrange("(co cp) -> cp co", cp=P)
                )

            def v_proj(b, xT, mts=None):
                for mt in (range(JC) if mts is None else mts):
                    psv2 = st_psum.tile([P, 2, IB], FP32, tag="st", name="v_ps")
                    ps_v = psv2[:, 0, :SH]
                    for ko in range(KO):
                        nc.tensor.matmul(
                            ps_v,
                            xT[:, ko, mt * P : (mt + 1) * P],
                            wv_sb[:, ko, :],
                            start=(ko == 0),
                            stop=(ko == KO - 1),
                        )
                    nc.vector.tensor_copy(
                        v_aug[:, b, mt, :, 0:DH],
                        ps_v.rearrange("p (h d) -> p h d", d=DH),
                    )

            def attention(h, b, between=None):
                """ST = k@q.T per j-chunk, exp on ACT, attn@V with the
                denominator in column DH.  All 4 i-sub accumulations of a
                block share one PSUM bank so the 'po' slots rotate once
                per block, not once per i-sub (normalizes are emitted
                after all 64 matmuls so the bank tracker doesn't
                interleave PE writes with DVE reads)."""
                po = h * DH
                for ib in range(NIB):
                    ptile = pt_pool.tile([P, JC, IB], BF16, tag="pt", name="ptile")
                    for jg in range(JC // 2):
                        ps_st = st_psum.tile(
                            [P, 2, IB], FP32, tag="st", name="st_ps"
                        )
                        for u in range(2):
                            jc = jg * 2 + u
                            nc.tensor.matmul(
                                ps_st[:, u, :],
                                kT[:, b, h, jc * P : (jc + 1) * P],
                                qT[:, b, h, ib * IB : (ib + 1) * IB],
                                start=True,
                                stop=True,
                            )
                        nc.scalar.activation(
                            ptile[:, jg * 2 : (jg + 1) * 2, :],
                            ps_st,
                            mybir.ActivationFunctionType.Exp,
                            scale=SCALE,
                        )
                    ps_o4 = o_psum.tile(
                        [P, ISUB, DH + 1], FP32, tag="po", name="o_ps"
                    )
                    for isub in range(ISUB):
                        for jc in range(JC):
                            nc.tensor.matmul(
                                ps_o4[:, isub, :],
                                ptile[:, jc, isub * P : (isub + 1) * P],
                                v_aug[:, b, jc, h, :],
                                start=(jc == 0),
                                stop=(jc == JC - 1),
                            )
                    for isub in range(ISUB):
                        ic = ib * ISUB + isub
                        recip = nrm.tile([P, 1], FP32, tag="recip", name="recip")
                        nc.vector.reciprocal(recip, ps_o4[:, isub, DH : DH + 1])
                        nc.vector.tensor_scalar_mul(
                            out_rows[:, b, ic, po : po + DH],
                            ps_o4[:, isub, 0:DH],
                            recip,
                        )
                    # block (h, b, ib) complete -> stage its A2A input
                    s = b * NIB + ib
                    nc.sync.dma_start(
                        a2a_ins[h][s],
                        out_rows[
                            :, b, ib * ISUB : (ib + 1) * ISUB, po : po + DH
                        ],
                    )
                    # interleaved lower-priority PE work (kept small so the
                    # scalar queue's ~8-ACT lookahead absorbs it)
                    if between is not None and ib < len(between):
                        between[ib]()

            def a2a_exchange(h):
                nc.gpsimd.collective_compute(
                    "AllToAll",
                    mybir.AluOpType.bypass,
                    replica_groups=REPLICA_GROUPS,
                    ins=[a2a_ins[h].opt()],
                    outs=[a2a_outs[h].opt()],
                )

            def receive(h):
                po = h * DH
                rstages = []
                for i in range(NCORES):
                    rstage = rst_pool.tile(
                        [P, NIB, DH], BF16, tag="rstage", name="rstage"
                    )
                    nc.scalar.dma_start(rstage, a2a_outs[h][i])
                    rstages.append(rstage)
                for i in range(NCORES):
                    rps = st_psum.tile([DH, NIB, P], BF16, tag="st", name="r_ps")
                    for q in range(NIB):
                        nc.tensor.transpose(
                            rps[:, q, :], rstages[i][:, q, :], ident_bf
                        )
                    nc.vector.tensor_copy(attnT[po : po + DH, i, :], rps)

            def final_projection():
                for cc in range(KO):
                    psf2 = st_psum.tile([P, 2, IB], FP32, tag="st", name="f_ps")
                    ps_f = psf2[:, 0, :ROWS]
                    for ko in range(KO):
                        nc.tensor.matmul(
                            ps_f,
                            wo_sb[:, ko, cc * P : (cc + 1) * P],
                            attnT[:, ko, :],
                            start=(ko == 0),
                            stop=(ko == KO - 1),
                        )
                    of = stage.tile([P, ROWS], FP32, tag="of", name="of")
                    nc.vector.tensor_scalar_add(of, ps_f, bias_sb[:, cc : cc + 1])
                    nc.sync.dma_start(out_ext[cc * P : (cc + 1) * P, :], of)

            # ---- program order chosen so exp starts early and the PE
            # always has lower-priority projection work to fill gaps ----
            load_qkv_weights()
            xT0 = load_xT(0)
            pe_warm(10)
            qk_proj(0, xT0)
            v_proj(0, xT0)
            xT1 = load_xT(1)
            load_out_weights()
            attention(0, 0, between=[
                lambda: qk_chunk(1, xT1, wk_sb, kT, range(NIB)),
                lambda: qk_chunk(1, xT1, wq_sb, qT, range(NIB)),
                lambda: v_proj(1, xT1, range(0, 8)),
                lambda: v_proj(1, xT1, range(8, JC)),
            ])
            attention(0, 1)
            a2a_exchange(0)        # flies under head-1 attention
            attention(1, 0)
            attention(1, 1)
            receive(0)             # PE work during the head-1 A2A flight
            a2a_exchange(1)
            pe_warm(45)            # PE activity across the A2A-1 flight
            receive(1)
            final_projection()

    nc.finalize()
    return nc


def _get_nc():
    if "nc" not in _NC_CACHE:
        _NC_CACHE["nc"] = _build()
    return _NC_CACHE["nc"]


def kernel(**inputs) -> np.ndarray:
    import os

    import ml_dtypes

    global LAST_RESULTS

    bf16 = ml_dtypes.bfloat16
    x = np.asarray(inputs["x"], dtype=np.float32)
    W_qkv = np.asarray(inputs["W_qkv"], dtype=np.float32)
    W_out = np.asarray(inputs["W_out"], dtype=np.float32)
    b_out = np.ascontiguousarray(np.asarray(inputs["b_out"], dtype=np.float32))

    x_bf = np.ascontiguousarray(
        x.transpose(0, 2, 1).reshape(B * DIM, N).astype(bf16)
    )
    wo_bf = np.ascontiguousarray(W_out.astype(bf16))
    wqkv_bf = W_qkv.astype(bf16)

    nc = _get_nc()

    in_maps = []
    for c in range(NCORES):
        in_maps.append(
            {
                "x": x_bf,
                "wq": np.ascontiguousarray(
                    wqkv_bf[:, 0 * INNER + c * SH : 0 * INNER + (c + 1) * SH]
                ),
                "wk": np.ascontiguousarray(
                    wqkv_bf[:, 1 * INNER + c * SH : 1 * INNER + (c + 1) * SH]
                ),
                "wv": np.ascontiguousarray(
                    wqkv_bf[:, 2 * INNER + c * SH : 2 * INNER + (c + 1) * SH]
                ),
                "wo": wo_bf,
                "bo": b_out,
            }
        )

    trace = os.environ.get("BASS_KERNEL_TRACE", "0") == "1"
    res = run_bass_kernel_spmd(
        nc, in_maps, core_ids=list(range(NCORES)), trace=trace
    )
    LAST_RESULTS = res

    y = np.empty((B, N, DIM), dtype=np.float32)
    for c in range(NCORES):
        b, r = c // 4, c % 4
        y[b, r * ROWS : (r + 1) * ROWS, :] = res.results[c]["out"].T
    return y

